# revision 26
# baseline (speedup 1.0000x reference)
"""Masked cross-attention kernel for Trainium2 (8 NeuronCores, SPMD).

Problem: B=16 batches of softmax(mask(Q@K^T/sqrt(D)))@V with
Lq=Lk=2048, D=DV=256.  The reference zeroes masked scores (NOT -inf)
before the softmax, so masked keys contribute exp(0)=1 to the
denominator and weight 1/denom on V rows.

KEY OPTIMIZATION vs the dense version: for keys k >= valid_length the
score is *exactly* 0 (K rows are host-zeroed), so every masked key
contributes exp(0)=1 * v_k to the numerator and 1 to the denominator.
A whole k-tile range [t*128, 2048) therefore collapses to a single
per-batch correction vector  corr = sum_{k>=t*128} [V|1][k]  that the
HOST precomputes exactly and FOLDS into the last (masked) V rows of
the computed range - those rows have attention weight exactly 1, so
adding corr there is exact and costs zero device instructions.  Only
ceil(valid_length/128) k-tiles of real matmul work remain per batch
(~half, for uniform valid_length).

SPMD constraint: one program runs on all 8 cores, so per-core work
must be structurally identical.  Work units are 64 jobs = (batch,
512-wide q tile) each needing r_b k-tiles.  Jobs are sorted by r
descending and dealt 8-per-slot into 8 slots; slot j executes
t_j = max r in slot on every core (padding tiles have zeroed K ->
exp(0)=1 -> exact masked behavior).  Sum t_j is within ~6% of the
ideal balance.  Slots execute smallest-first so the first DMA is
tiny (fast start).

Per slot, per core: stage 1: S^T[k,q] in PSUM (Kt.T @ Qt), exp via
ScalarE (scale=1/16 folded) -> P^T bf16; stage 2: O[q,v] = (P^T).T @
[V|1] accumulated over k tiles, divide by column 256.  Stage-2
subtile chains of slot e-1 are interleaved BETWEEN stage-1 exp groups
of slot e so the PE always has queued work while ScalarE's exp chain
(the 2nd-busiest engine) catches up.

DMA: segments are enumerated in need order and greedily split across
the two HWDGE rings (sync + scalar engines, ~170 GB/s each,
FIFO-serial per ring) so aggregate streaming tracks consumption.
Outputs are batched one DMA per job ([128, 4*256] fp16, strided DRAM
view) to cut descriptor-write cost on the issuing engines.
"""

import numpy as np
import ml_dtypes

import concourse.bass as bass
import concourse.mybir as mybir
import concourse.tile as tile
from concourse import bacc
from concourse.bass_utils import run_bass_kernel_spmd

B, LQ, LK, D, DV = 16, 2048, 2048, 256, 256
N_CORES = 8

QT = 512            # q-tile width (stage-1 moving free dim)
NQT = LQ // QT      # 4 q tiles per batch
KT = 128            # k-tile (partition dim of S^T)
NKT = LK // KT      # 16 k tiles max
NDC = D // 128      # contraction chunks (2)
QS = 128            # q-subtile for stage 2
NQS = QT // QS      # 4
VF = DV + 1         # 257: V plus the ones column
NSLOT = 8           # jobs per core == slots
WARMUP_MMS = 8      # HAM warm-up matmuls bridging the initial DMA wait

_BF16 = mybir.dt.bfloat16
_F16 = mybir.dt.float16
_F32 = mybir.dt.float32

_NC_CACHE = {}


def _schedule(vl):
    """Per-batch tile counts, the common slot profile (ascending
    execution order) and the per-core job assignment."""
    r = []
    for b in range(B):
        v = int(vl[b])
        rb = max(1, -(-v // KT))          # ceil(v/128), min 1
        # Guarantee >=2 masked rows inside the computed range whenever a
        # suffix correction exists (rb < NKT), so the correction can be
        # folded into masked V rows as a bf16 hi/lo pair.
        if rb < NKT and rb * KT - v < 2:
            rb += 1
        r.append(rb)
    jobs = [(r[b], b, qi) for b in range(B) for qi in range(NQT)]
    jobs.sort(key=lambda x: (-x[0], x[1], x[2]))
    profile = [0] * NSLOT
    assign = [[None] * NSLOT for _ in range(N_CORES)]
    for j in range(NSLOT):
        grp = jobs[N_CORES * j:N_CORES * (j + 1)]
        e = NSLOT - 1 - j                  # execution index (ascending t)
        profile[e] = max(1, grp[0][0])
        for c in range(N_CORES):
            assign[c][e] = (grp[c][1], grp[c][2])
    return tuple(profile), assign, r


def _npair(t):
    return -(-t // 2)


def _chunk_cols(profile, nm):
    e = int(nm[1:-1]) if nm[-1] in "ht" else int(nm[1:])
    if nm[0] == "k":
        np_ = _npair(profile[e])
        split = (e == _visit(profile)[0] and np_ > 2)
        if nm[-1] == "h":
            return 2 * 512 if split else np_ * 512
        return (np_ - 2) * 512
    if nm[0] == "q":
        return NDC * QT
    return profile[e] * VF


def _visit(profile):
    """Slot execution order: ascending t rotated left by one, so the
    start streams tiny DMAs and the LAST slot is the smallest (short
    post-PE tail)."""
    order = sorted(range(NSLOT), key=lambda e: (profile[e], e))
    return order[1:] + order[:1]


def _segments(profile):
    """DMA segments: K+Q on the sync ring, V on the scalar ring (the
    scalar ENGINE also runs exp, so its descriptor writes are deferred
    in the emission schedule, not here).  Blob layout follows this
    order; a multi-chunk segment is stored partition-major over the
    WHOLE segment."""
    v0 = _visit(profile)[0]
    segs = []
    for e in range(NSLOT):
        if e == v0 and _npair(profile[e]) > 2:
            segs.append(("sync", [f"k{e}h", f"q{e}"]))
            segs.append(("sync", [f"k{e}t"]))
        else:
            segs.append(("sync", [f"k{e}h", f"q{e}"]))
        segs.append(("scalar", [f"v{e}"]))
    return segs


def _layout(profile):
    """Blob offsets per chunk, in segment order."""
    segs = _segments(profile)
    off = {}
    o = 0
    for _ring, names in segs:
        for nm in names:
            n = _chunk_cols(profile, nm)
            off[nm] = (o, n)
            o += n
    return segs, off, o


def _build_nc(profile):
    segs, off, tot = _layout(profile)

    nc = bacc.Bacc("TRN2", target_bir_lowering=False, debug=False,
                   num_devices=N_CORES)
    blob_d = nc.declare_dram_parameter("blob", [128 * tot], _BF16,
                                       isOutput=False)
    out_d = nc.declare_dram_parameter("out", [NSLOT, QT, DV], _F16,
                                      isOutput=True)
    tmax = max(profile)

    with tile.TileContext(nc) as tc:
        with (
            tc.tile_pool(name="seg", bufs=1) as seg_pool,
            tc.tile_pool(name="p", bufs=2) as p_pool,
            tc.tile_pool(name="osb", bufs=3) as o_pool,
            tc.tile_pool(name="small", bufs=8) as small_pool,
            tc.tile_pool(name="ps_s", bufs=2, space="PSUM") as ps_s,
            tc.tile_pool(name="ps_o", bufs=4, space="PSUM") as ps_o,
        ):
            tiles = {}

            def load(names, engine):
                lo = off[names[0]][0]
                n = sum(off[nm][1] for nm in names)
                t = seg_pool.tile([128, n], _BF16, tag="+".join(names))
                src = blob_d[128 * lo:128 * (lo + n)].rearrange(
                    "(p n) -> p n", p=128)
                engine.dma_start(out=t, in_=src)
                for nm in names:
                    tiles[nm] = (t, off[nm][0] - lo)

            def chunk_slice(nm, a, b):
                t, o = tiles[nm]
                return t[:, o + a:o + b]

            def kt_slice(e, c, kj):
                pair, half = kj // 2, kj % 2
                col = pair * 512 + c * 256 + half * 128
                hcols = off[f"k{e}h"][1]
                if col < hcols:
                    return chunk_slice(f"k{e}h", col, col + 128)
                return chunk_slice(f"k{e}t", col - hcols, col - hcols + 128)

            def qt_slice(e, c):
                return chunk_slice(f"q{e}", c * QT, (c + 1) * QT)

            def v1_slice(e, kj):
                return chunk_slice(f"v{e}", kj * VF, (kj + 1) * VF)

            kq_segs, v_segs = {}, {}
            for ring, names in segs:
                e = int(names[0][1:-1] if names[0][0] == "k"
                        else names[0][1:])
                if names[0][0] == "k":
                    kq_segs.setdefault(e, []).append(names)
                else:
                    v_segs[e] = names

            visit = _visit(profile)

            def issue_kq(e):
                for names in kq_segs[e]:
                    load(names, nc.sync)

            def issue_v(e):
                load(v_segs[e], nc.scalar)

            # Prologue: only the first slots' segments.  Later segments
            # are issued INSIDE the emission stream (after each slot's
            # first exp) so the scalar engine's descriptor writes never
            # delay the early exp chain, and the sync ring streams K/Q
            # a few slots ahead of consumption.
            for i in range(min(3, NSLOT)):
                issue_kq(visit[i])

            # HAM warm-up: matmuls on an UNINITIALIZED tile (no producer ->
            # no waits -> PE starts right after its preamble) into a
            # throwaway PSUM group.  The tiny DVE read keeps DCE away.
            warm = small_pool.tile([128, QT], _BF16, tag="warm")
            wps = ps_o.tile([128, QT], _F32, tag="o", name="warm_ps")
            for w in range(WARMUP_MMS):
                nc.tensor.matmul(wps, lhsT=warm[:, :128], rhs=warm,
                                 start=(w == 0), stop=(w == WARMUP_MMS - 1))
            nc.vector.tensor_copy(out=warm[:, 0:1], in_=wps[:, 0:1])

            out_engines = [nc.sync, nc.scalar]

            def s2_unit(e, p_sb, s, o_job, out_eng, split_out):
                """One stage-2 q-subtile: accumulate over k tiles,
                normalize, and DMA (batched per job; per-subtile for the
                final slot so the tail DMA overlaps the last chains)."""
                t = profile[e]
                o_ps = ps_o.tile([128, VF], _F32, tag="o")
                for kj in range(t):
                    nc.tensor.matmul(
                        o_ps,
                        lhsT=p_sb[:, kj * QT + s * QS:kj * QT + (s + 1) * QS],
                        rhs=v1_slice(e, kj),
                        start=(kj == 0), stop=(kj == t - 1),
                    )
                recip = small_pool.tile([128, 1], _F32, tag="r")
                nc.vector.reciprocal(out=recip, in_=o_ps[:, DV:DV + 1])
                nc.vector.tensor_scalar_mul(
                    out=o_job[:, s * DV:(s + 1) * DV], in0=o_ps[:, :DV],
                    scalar1=recip)
                if split_out:
                    out_eng.dma_start(
                        out=out_d[e, s * QS:(s + 1) * QS, :],
                        in_=o_job[:, s * DV:(s + 1) * DV])
                elif s == NQS - 1:
                    dst = out_d[e].rearrange("(s p) v -> p s v", s=NQS, p=QS)
                    src = o_job.rearrange("p (s v) -> p s v", s=NQS)
                    out_eng.dma_start(out=dst, in_=src)

            pending = None  # (e, p_sb, o_job, next_subtile, out_eng)
            for i, e in enumerate(visit):
                t = profile[e]
                npair = _npair(t)
                # spread the previous slot's 4 stage-2 units evenly over
                # this slot's stage-1 groups so the PE stays fed through
                # the whole exp chain, not just its start
                step = max(1, npair // (NQS + 1))
                slots_at = set(range(1, npair, step))
                p_sb = p_pool.tile([128, tmax * QT], _BF16, tag="p")
                for g in range(npair):
                    w = 2 if 2 * g + 1 < t else 1
                    ps = ps_s.tile([128, 2 * QT], _F32, tag="s")
                    for h in range(w):
                        kj = 2 * g + h
                        for c in range(NDC):
                            nc.tensor.matmul(
                                ps[:, h * QT:(h + 1) * QT],
                                lhsT=kt_slice(e, c, kj),
                                rhs=qt_slice(e, c),
                                start=(c == 0),
                                stop=(c == NDC - 1),
                            )
                    nc.scalar.activation(
                        out=p_sb[:, g * 2 * QT:g * 2 * QT + w * QT],
                        in_=ps[:, :w * QT],
                        func=mybir.ActivationFunctionType.Exp,
                        scale=1.0 / 16.0)
                    if g == 0:
                        if i == 0:
                            # V descriptors deferred past exp(0) so the
                            # first exp is ScalarE's first instruction
                            for ii in range(min(2, NSLOT)):
                                issue_v(visit[ii])
                        if i + 3 < NSLOT:
                            issue_kq(visit[i + 3])
                        if i + 2 < NSLOT:
                            issue_v(visit[i + 2])
                    # keep the PE fed while ScalarE works through exp:
                    # the previous slot's stage-2 subtile chains slot
                    # between stage-1 groups
                    elif (pending is not None and pending[3] < NQS
                          and g in slots_at):
                        ep, pp, oj, si, oe = pending
                        s2_unit(ep, pp, si, oj, oe, ep == visit[-2])
                        pending = (ep, pp, oj, si + 1, oe)
                if pending is not None:
                    ep, pp, oj, si, oe = pending
                    for s in range(si, NQS):
                        s2_unit(ep, pp, s, oj, oe, ep == visit[-2])
                o_job = o_pool.tile([128, NQS * DV], _F16, tag="o_job")
                pending = (e, p_sb, o_job, 0, out_engines[i % 2])
            ep, pp, oj, si, oe = pending
            for s in range(si, NQS):
                s2_unit(ep, pp, s, oj, oe, True)

    nc.compile()
    return nc


def _get_nc(profile):
    if profile not in _NC_CACHE:
        _NC_CACHE[profile] = _build_nc(profile)
    return _NC_CACHE[profile]


def _prepare(query, key, value, valid_length):
    query = np.asarray(query, dtype=np.float32)
    key = np.asarray(key, dtype=np.float32)
    value = np.asarray(value, dtype=np.float32)
    vl = np.asarray(valid_length).astype(np.int64)

    profile, assign, r = _schedule(vl)
    segs, off, tot = _layout(profile)

    bf16 = ml_dtypes.bfloat16

    kz = key.copy()
    for b in range(B):
        kz[b, int(vl[b]):, :] = 0.0
    kzT = kz.transpose(0, 2, 1).reshape(B, NDC, 128, LK)
    qT = query.transpose(0, 2, 1).reshape(B, NDC, 128, LQ)
    v1 = np.concatenate(
        [value, np.ones((B, LK, 1), np.float32)], axis=-1)  # [B, LK, VF]
    # suffix sums at tile boundaries: suf[b, m] = sum_{k >= m*128} v1[b, k]
    blk = v1.reshape(B, NKT, KT, VF).sum(axis=2)            # [B, 16, VF]
    suf = np.zeros((B, NKT + 1, VF), np.float32)
    suf[:, :NKT] = blk[:, ::-1].cumsum(axis=1)[:, ::-1]

    def k_chunk(b, t):
        np_ = _npair(t)
        kp = np.zeros((NDC, 128, np_ * 256), np.float32)
        kp[:, :, :t * KT] = kzT[b][:, :, :t * KT]
        arr = kp.reshape(NDC, 128, np_, 256).transpose(1, 2, 0, 3)
        return arr.reshape(128, np_ * 512)

    def q_chunk(b, qi):
        return qT[b][:, :, qi * QT:(qi + 1) * QT] \
            .transpose(1, 0, 2).reshape(128, NDC * QT)

    def v_chunk(b, t):
        vj = v1[b, :t * KT].copy()                          # [t*128, VF]
        if t < NKT:
            # fold the exact masked-suffix correction into the last two
            # rows of the computed range (both masked -> weight exactly 1)
            T = vj[-1] + vj[-2] + suf[b, t]
            hi = T.astype(bf16).astype(np.float32)
            vj[-1] = hi
            vj[-2] = T - hi
        return vj.reshape(t, 128, VF).transpose(1, 0, 2).reshape(128, t * VF)

    blobs = []
    for c in range(N_CORES):
        parts = {}
        for e, t in enumerate(profile):
            b, qi = assign[c][e]
            kc = k_chunk(b, t)
            if f"k{e}t" in off:
                parts[f"k{e}h"] = kc[:, :2 * 512]
                parts[f"k{e}t"] = kc[:, 2 * 512:]
            else:
                parts[f"k{e}h"] = kc
            parts[f"q{e}"] = q_chunk(b, qi)
            parts[f"v{e}"] = v_chunk(b, t)
        flat_segs = []
        for _ring, names in segs:
            seg = (np.concatenate([parts[nm] for nm in names], axis=1)
                   if len(names) > 1 else parts[names[0]])
            flat_segs.append(seg.astype(bf16).reshape(-1))
        flat = np.concatenate(flat_segs)
        assert flat.shape == (128 * tot,)
        blobs.append(flat)
    return blobs, profile, assign


def _run(inputs, trace=False):
    blobs, profile, assign = _prepare(**inputs)
    in_maps = [{"blob": blobs[c]} for c in range(N_CORES)]
    nc = _get_nc(profile)
    res = run_bass_kernel_spmd(nc, in_maps, core_ids=list(range(N_CORES)),
                               trace=trace)
    out = np.empty((B, LQ, DV), np.float32)
    for c in range(N_CORES):
        r_c = np.asarray(res.results[c]["out"], dtype=np.float32)
        for e in range(NSLOT):
            b, qi = assign[c][e]
            out[b, qi * QT:(qi + 1) * QT] = r_c[e]
    return out, res


def kernel(query, key, value, valid_length):
    out, _ = _run(dict(query=query, key=key, value=value,
                       valid_length=valid_length))
    return out
